# revision 5
# baseline (speedup 1.0000x reference)
"""EpiNet (episodic memory recall) Trainium2 kernel, 8 NeuronCores.

Strategy:
  - c_buffer row-sharded 50000 rows/core. Host folds the per-row scale
    sal_r * rsqrt(||c_r||^2 + eps) into the c rows and transposes to
    [128, 50176] (latent on partitions) so the device streams it straight
    into the PE as the moving matmul operand. z's own l2 normalization is a
    per-batch-row positive factor, invariant for top-k and (up to the 1e-8
    eps) for the weighted recall sum, so it is skipped entirely.
  - z_buffer column-sharded [400000, 16] per core: any global top-k index
    is gatherable locally via indirect DMA; each core produces its 16-col
    slice of r and one AllGather assembles the full r.
  - Encoder/decoder replicated on every core (tiny).
  - Scores: PE matmul with two stationary operands zT|0 and 0|zT so two
    1792-row chunks land in one [128, 1792] PSUM tile (all 128 partitions
    carry batch rows). DVE max/max_index (hw top-8) per pair of chunks.
  - Two tiny collectives: AllGather of (val, idx) candidates, AllGather of
    r slices.
"""

import os
import sys

import numpy as np

sys.path.insert(0, "/opt/trn_rl_repo")

from concourse import bacc, bass, mybir
import concourse.tile as tile
from concourse.bass_utils import run_bass_kernel_spmd

F32 = mybir.dt.float32
U32 = mybir.dt.uint32

B = 64
DIN = 784
EHID = 256
D = 128  # latent
HID = 256
NCLS = 100
CAP = 400000
TOPK = 8
EPS = 1e-8
DECAY = 0.01

NCORES = 8
RLOC = CAP // NCORES          # 50000 rows per core
RPAD = 50176                  # 392 * 128 = 14 * 3584
F = 1792                      # chunk free size (4 matmuls of 448)
NPAIR = RPAD // (2 * F)       # 14 pairs of chunks
NSLOT = NPAIR * 8             # 112 candidate slots per partition

LAST_RUN = None  # BassKernelResults of the most recent execution


def _build_program() -> bass.Bass:
    # Bacc (not raw Bass): its compile() runs generate_event_semaphores,
    # which legalizes multi-wait instructions for the TRN2 1-wait limit.
    nc = bacc.Bacc(num_devices=NCORES)
    dp = nc.declare_dram_parameter

    ct_d = dp("ct", [128, RPAD], F32, isOutput=False)
    z_d = dp("zcol", [CAP, 16], F32, isOutput=False)
    x_d = dp("x", [B, DIN], F32, isOutput=False)
    w1_d = dp("w1", [DIN, EHID], F32, isOutput=False)
    b1_d = dp("b1", [128, 2], F32, isOutput=False)
    w2_d = dp("w2", [EHID, D], F32, isOutput=False)
    b2_d = dp("b2", [128, 1], F32, isOutput=False)
    dw1_d = dp("dw1", [2 * D, HID], F32, isOutput=False)
    db1_d = dp("db1", [128, 2], F32, isOutput=False)
    dw2_d = dp("dw2", [HID, NCLS], F32, isOutput=False)
    db2_d = dp("db2", [NCLS, 1], F32, isOutput=False)
    id_d = dp("ident", [128, 128], F32, isOutput=False)
    io112_d = dp("iota112", [128, NSLOT], F32, isOutput=False)
    bm112_d = dp("base112", [128, NSLOT], F32, isOutput=False)
    offs_d = dp("offs", [128, 1], F32, isOutput=False)
    io128_d = dp("iota128", [B, 128], F32, isOutput=False)
    out_d = dp("logits", [B, NCLS], F32, isOutput=True)

    AF = mybir.ActivationFunctionType
    OP = mybir.AluOpType

    with tile.TileContext(nc) as tc:
        with (
            tc.tile_pool(name="pers", bufs=1) as pers,
            tc.tile_pool(name="ct", bufs=6) as ctp,
            tc.tile_pool(name="sc", bufs=3) as scp,
            tc.tile_pool(name="small", bufs=2) as sm,
            tc.tile_pool(name="ps", bufs=2, space="PSUM") as psp,
            tc.tile_pool(name="dram", bufs=1, space="DRAM") as dram,
        ):
            # ---- constants / params to SBUF ----
            ident = pers.tile([128, 128], F32, tag="ident")
            nc.sync.dma_start(out=ident[:], in_=id_d[:])
            io112 = pers.tile([128, NSLOT], F32, tag="io112")
            nc.sync.dma_start(out=io112[:], in_=io112_d[:])
            bm112 = pers.tile([128, NSLOT], F32, tag="bm112")
            nc.sync.dma_start(out=bm112[:], in_=bm112_d[:])
            offs = pers.tile([128, 1], F32, tag="offs")
            nc.sync.dma_start(out=offs[:], in_=offs_d[:])
            io128 = pers.tile([B, 128], F32, tag="io128")
            nc.sync.dma_start(out=io128[:], in_=io128_d[:])

            b1 = pers.tile([128, 2], F32, tag="b1")
            nc.sync.dma_start(out=b1[:], in_=b1_d[:])
            b2 = pers.tile([128, 1], F32, tag="b2")
            nc.sync.dma_start(out=b2[:], in_=b2_d[:])
            db1 = pers.tile([128, 2], F32, tag="db1")
            nc.sync.dma_start(out=db1[:], in_=db1_d[:])
            db2 = pers.tile([128, 1], F32, tag="db2")
            nc.sync.dma_start(out=db2[0:NCLS, :], in_=db2_d[:])

            w1 = pers.tile([112, 7 * EHID], F32, tag="w1")
            for k in range(7):
                nc.sync.dma_start(
                    out=w1[:, k * EHID:(k + 1) * EHID],
                    in_=w1_d[k * 112:(k + 1) * 112, :],
                )
            w2 = pers.tile([128, 2 * D], F32, tag="w2")
            for k in range(2):
                nc.sync.dma_start(
                    out=w2[:, k * D:(k + 1) * D],
                    in_=w2_d[k * 128:(k + 1) * 128, :],
                )
            dw1 = pers.tile([128, 2 * HID], F32, tag="dw1")
            for k in range(2):
                nc.sync.dma_start(
                    out=dw1[:, k * HID:(k + 1) * HID],
                    in_=dw1_d[k * 128:(k + 1) * 128, :],
                )
            dw2 = pers.tile([128, 2 * NCLS], F32, tag="dw2")
            for k in range(2):
                nc.sync.dma_start(
                    out=dw2[:, k * NCLS:(k + 1) * NCLS],
                    in_=dw2_d[k * 128:(k + 1) * 128, :],
                )

            # ---- encoder: z^T [128, 64] ----
            x_sb = pers.tile([B, DIN], F32, tag="x")
            nc.sync.dma_start(out=x_sb[:], in_=x_d[:])

            xT = pers.tile([112, 7 * B], F32, tag="xT")
            for k in range(7):
                ps_x = psp.tile([128, 2048], F32, tag="ps")
                nc.tensor.transpose(
                    out=ps_x[0:112, 0:B],
                    in_=x_sb[:, k * 112:(k + 1) * 112],
                    identity=ident[0:B, 0:B],
                )
                nc.scalar.activation(
                    out=xT[:, k * B:(k + 1) * B], in_=ps_x[0:112, 0:B], func=AF.Copy
                )

            hT = pers.tile([128, 2 * B], F32, tag="hT")
            for m in range(2):
                ps_h = psp.tile([128, 2048], F32, tag="ps")
                for k in range(7):
                    nc.tensor.matmul(
                        ps_h[:, 0:B],
                        lhsT=w1[:, k * EHID + m * 128:k * EHID + (m + 1) * 128],
                        rhs=xT[:, k * B:(k + 1) * B],
                        start=(k == 0),
                        stop=(k == 6),
                    )
                nc.scalar.activation(
                    out=hT[:, m * B:(m + 1) * B],
                    in_=ps_h[:, 0:B],
                    func=AF.Relu,
                    bias=b1[:, m:m + 1],
                )

            ps_z = psp.tile([128, 2048], F32, tag="ps")
            for k in range(2):
                nc.tensor.matmul(
                    ps_z[:, 0:B],
                    lhsT=w2[:, k * D:(k + 1) * D],
                    rhs=hT[:, k * B:(k + 1) * B],
                    start=(k == 0),
                    stop=(k == 1),
                )
            zT = pers.tile([128, B], F32, tag="zT")
            nc.vector.tensor_scalar(
                out=zT[:], in0=ps_z[:, 0:B], scalar1=b2[:, 0:1], scalar2=None,
                op0=OP.add,
            )

            # stationary operands: zT in cols 0:64 -> out partitions 0:64;
            # zT in cols 64:128 -> out partitions 64:128
            padA = pers.tile([128, 128], F32, tag="padA")
            nc.vector.memset(padA[:], 0.0)
            nc.vector.tensor_copy(out=padA[:, 0:B], in_=zT[:])
            padB = pers.tile([128, 128], F32, tag="padB")
            nc.vector.memset(padB[:], 0.0)
            nc.vector.tensor_copy(out=padB[:, B:128], in_=zT[:])

            # ---- main loop: scores + streaming hw top-8 ----
            V = pers.tile([128, NSLOT], F32, tag="V")
            I = pers.tile([128, NSLOT], U32, tag="I")
            for p in range(NPAIR):
                ct = ctp.tile([128, 2 * F], F32, tag="ct")
                nc.sync.dma_start(
                    out=ct[:], in_=ct_d[:, p * 2 * F:(p + 1) * 2 * F]
                )
                ps = psp.tile([128, 2048], F32, tag="ps")
                for i in range(4):
                    nc.tensor.matmul(
                        ps[:, i * 512:i * 512 + 448],
                        lhsT=padA,
                        rhs=ct[:, i * 448:(i + 1) * 448],
                        start=True,
                        stop=False,
                    )
                    nc.tensor.matmul(
                        ps[:, i * 512:i * 512 + 448],
                        lhsT=padB,
                        rhs=ct[:, F + i * 448:F + (i + 1) * 448],
                        start=False,
                        stop=True,
                    )
                sc = scp.tile([128, F], F32, tag="sc")
                for i in range(4):
                    nc.scalar.activation(
                        out=sc[:, i * 448:(i + 1) * 448],
                        in_=ps[:, i * 512:i * 512 + 448],
                        func=AF.Copy,
                    )
                nc.vector.max(out=V[:, p * 8:p * 8 + 8], in_=sc[:])
                nc.vector.max_index(
                    out=I[:, p * 8:p * 8 + 8],
                    in_max=V[:, p * 8:p * 8 + 8],
                    in_values=sc[:],
                )

            # ---- per-core top-8 per partition-half + index resolution ----
            gv8 = pers.tile([128, 8], F32, tag="gv8")
            nc.vector.max(out=gv8[:], in_=V[:])
            gs8u = pers.tile([128, 8], U32, tag="gs8u")
            nc.vector.max_index(out=gs8u[:], in_max=gv8[:], in_values=V[:])
            gs8f = pers.tile([128, 8], F32, tag="gs8f")
            nc.vector.tensor_copy(out=gs8f[:], in_=gs8u[:])
            If = pers.tile([128, NSLOT], F32, tag="If")
            nc.vector.tensor_copy(out=If[:], in_=I[:])
            BI = pers.tile([128, NSLOT], F32, tag="BI")
            nc.vector.tensor_tensor(out=BI[:], in0=bm112[:], in1=If[:], op=OP.add)

            gidx = pers.tile([128, 8], F32, tag="gidx")
            for k in range(8):
                mask = sm.tile([128, NSLOT], F32, tag="mask")
                nc.vector.tensor_scalar(
                    out=mask[:], in0=io112[:], scalar1=gs8f[:, k:k + 1],
                    scalar2=None, op0=OP.is_equal,
                )
                nc.vector.tensor_tensor(
                    out=mask[:], in0=mask[:], in1=BI[:], op=OP.mult
                )
                nc.vector.tensor_reduce(
                    out=gidx[:, k:k + 1], in_=mask[:],
                    axis=mybir.AxisListType.X, op=OP.add,
                )
            # local row -> global row (adds half-offset 1792 for p>=64 and
            # core_id * 50000)
            nc.vector.tensor_scalar(
                out=gidx[:], in0=gidx[:], scalar1=offs[:, 0:1], scalar2=None,
                op0=OP.add,
            )

            # ---- fold halves into [64, 16] candidates, exchange ----
            cand = pers.tile([B, 32], F32, tag="cand")
            nc.gpsimd.dma_start(out=cand[:, 0:8], in_=gv8[0:B, :])
            nc.gpsimd.dma_start(out=cand[:, 8:16], in_=gv8[B:128, :])
            nc.gpsimd.dma_start(out=cand[:, 16:24], in_=gidx[0:B, :])
            nc.gpsimd.dma_start(out=cand[:, 24:32], in_=gidx[B:128, :])

            cin = dram.tile([B, 32], F32)
            cout = dram.tile([B * NCORES, 32], F32)
            nc.gpsimd.dma_start(out=cin[:], in_=cand[:])
            nc.gpsimd.collective_compute(
                "AllGather",
                OP.bypass,
                replica_groups=[list(range(NCORES))],
                ins=[cin.opt()],
                outs=[cout.opt()],
            )

            va = pers.tile([B, 128], F32, tag="va")
            ia = pers.tile([B, 128], F32, tag="ia")
            for c in range(NCORES):
                nc.sync.dma_start(
                    out=va[:, c * 16:(c + 1) * 16],
                    in_=cout[c * B:(c + 1) * B, 0:16],
                )
                nc.sync.dma_start(
                    out=ia[:, c * 16:(c + 1) * 16],
                    in_=cout[c * B:(c + 1) * B, 16:32],
                )

            # ---- global top-8 of 128 candidates ----
            gv = pers.tile([B, 8], F32, tag="gv")
            nc.vector.max(out=gv[:], in_=va[:])
            gslu = pers.tile([B, 8], U32, tag="gslu")
            nc.vector.max_index(out=gslu[:], in_max=gv[:], in_values=va[:])
            gslf = pers.tile([B, 8], F32, tag="gslf")
            nc.vector.tensor_copy(out=gslf[:], in_=gslu[:])

            gfin = pers.tile([B, 8], F32, tag="gfin")
            for k in range(8):
                mask2 = sm.tile([B, 128], F32, tag="mask2")
                nc.vector.tensor_scalar(
                    out=mask2[:], in0=io128[:], scalar1=gslf[:, k:k + 1],
                    scalar2=None, op0=OP.is_equal,
                )
                nc.vector.tensor_tensor(
                    out=mask2[:], in0=mask2[:], in1=ia[:], op=OP.mult
                )
                nc.vector.tensor_reduce(
                    out=gfin[:, k:k + 1], in_=mask2[:],
                    axis=mybir.AxisListType.X, op=OP.add,
                )
            gfinu = pers.tile([B, 8], U32, tag="gfinu")
            nc.vector.tensor_copy(out=gfinu[:], in_=gfin[:])

            # ---- weighted recall sum on this core's 16 z columns ----
            den = pers.tile([B, 1], F32, tag="den")
            nc.vector.tensor_reduce(
                out=den[:], in_=gv[:], axis=mybir.AxisListType.X, op=OP.add
            )
            nc.vector.tensor_scalar(
                out=den[:], in0=den[:], scalar1=float(EPS), scalar2=None,
                op0=OP.add,
            )
            rden = pers.tile([B, 1], F32, tag="rden")
            nc.vector.reciprocal(out=rden[:], in_=den[:])

            racc = pers.tile([B, 16], F32, tag="racc")
            for k in range(8):
                zk = sm.tile([B, 16], F32, tag="zk")
                nc.gpsimd.indirect_dma_start(
                    out=zk[:],
                    out_offset=None,
                    in_=z_d[:],
                    in_offset=bass.IndirectOffsetOnAxis(
                        ap=gfinu[:, k:k + 1], axis=0
                    ),
                )
                if k == 0:
                    nc.vector.tensor_scalar(
                        out=racc[:], in0=zk[:], scalar1=gv[:, 0:1],
                        scalar2=None, op0=OP.mult,
                    )
                else:
                    tmp = sm.tile([B, 16], F32, tag="tmp")
                    nc.vector.tensor_scalar(
                        out=tmp[:], in0=zk[:], scalar1=gv[:, k:k + 1],
                        scalar2=None, op0=OP.mult,
                    )
                    nc.vector.tensor_tensor(
                        out=racc[:], in0=racc[:], in1=tmp[:], op=OP.add
                    )
            nc.vector.tensor_scalar(
                out=racc[:], in0=racc[:], scalar1=rden[:, 0:1], scalar2=None,
                op0=OP.mult,
            )

            rin = dram.tile([B, 16], F32)
            rout = dram.tile([B * NCORES, 16], F32)
            nc.gpsimd.dma_start(out=rin[:], in_=racc[:])
            nc.gpsimd.collective_compute(
                "AllGather",
                OP.bypass,
                replica_groups=[list(range(NCORES))],
                ins=[rin.opt()],
                outs=[rout.opt()],
            )
            r_sb = pers.tile([B, 128], F32, tag="r_sb")
            for c in range(NCORES):
                nc.sync.dma_start(
                    out=r_sb[:, c * 16:(c + 1) * 16],
                    in_=rout[c * B:(c + 1) * B, :],
                )

            # ---- decoder ----
            ps_rT = psp.tile([128, 2048], F32, tag="ps")
            nc.tensor.transpose(
                out=ps_rT[:, 0:B], in_=r_sb[:], identity=ident[0:B, 0:B]
            )
            rT = pers.tile([128, B], F32, tag="rT")
            nc.vector.tensor_copy(out=rT[:], in_=ps_rT[:, 0:B])

            hdT = pers.tile([128, 2 * B], F32, tag="hdT")
            for m in range(2):
                ps_hd = psp.tile([128, 2048], F32, tag="ps")
                nc.tensor.matmul(
                    ps_hd[:, 0:B],
                    lhsT=dw1[:, 0 * HID + m * 128:0 * HID + (m + 1) * 128],
                    rhs=zT[:],
                    start=True,
                    stop=False,
                )
                nc.tensor.matmul(
                    ps_hd[:, 0:B],
                    lhsT=dw1[:, 1 * HID + m * 128:1 * HID + (m + 1) * 128],
                    rhs=rT[:],
                    start=False,
                    stop=True,
                )
                nc.scalar.activation(
                    out=hdT[:, m * B:(m + 1) * B],
                    in_=ps_hd[:, 0:B],
                    func=AF.Relu,
                    bias=db1[:, m:m + 1],
                )

            ps_lg = psp.tile([128, 2048], F32, tag="ps")
            for m in range(2):
                nc.tensor.matmul(
                    ps_lg[0:NCLS, 0:B],
                    lhsT=dw2[:, m * NCLS:(m + 1) * NCLS],
                    rhs=hdT[:, m * B:(m + 1) * B],
                    start=(m == 0),
                    stop=(m == 1),
                )
            lgT = pers.tile([128, B], F32, tag="lgT")
            nc.vector.tensor_scalar(
                out=lgT[0:NCLS, :], in0=ps_lg[0:NCLS, 0:B],
                scalar1=db2[0:NCLS, 0:1], scalar2=None, op0=OP.add,
            )

            ps_out = psp.tile([128, 2048], F32, tag="ps")
            nc.tensor.transpose(
                out=ps_out[0:B, 0:NCLS],
                in_=lgT[0:NCLS, :],
                identity=ident[0:NCLS, 0:NCLS],
            )
            out_sb = pers.tile([B, NCLS], F32, tag="out_sb")
            nc.scalar.activation(
                out=out_sb[:], in_=ps_out[0:B, 0:NCLS], func=AF.Copy
            )
            nc.sync.dma_start(out=out_d[:], in_=out_sb[:])

    nc.compile()
    return nc


_NC_CACHE = None


def _get_program():
    global _NC_CACHE
    if _NC_CACHE is None:
        _NC_CACHE = _build_program()
    return _NC_CACHE


def kernel(**inputs) -> np.ndarray:
    global LAST_RUN
    c = np.ascontiguousarray(np.asarray(inputs["c_buffer"], dtype=np.float32))
    zb = np.ascontiguousarray(np.asarray(inputs["z_buffer"], dtype=np.float32))
    r0 = np.asarray(inputs["r0_buffer"], dtype=np.float32)
    tau = np.asarray(inputs["tau_buffer"], dtype=np.float32)
    x = np.asarray(inputs["x"], dtype=np.float32)

    # fold salience * rsqrt(||c||^2 + eps) into c rows (f32 math, matching
    # the reference's f32 pipeline closely)
    sal = r0 * np.exp(np.float32(-DECAY) * tau)
    norm2 = np.einsum("nd,nd->n", c, c, dtype=np.float32).astype(np.float32)
    scale = sal / np.sqrt(norm2 + np.float32(EPS))

    common = {
        "x": x,
        "w1": np.ascontiguousarray(np.asarray(inputs["enc_w1"], np.float32)),
        "b1": np.ascontiguousarray(
            np.asarray(inputs["enc_b1"], np.float32).reshape(2, 128).T
        ),
        "w2": np.ascontiguousarray(np.asarray(inputs["enc_w2"], np.float32)),
        "b2": np.ascontiguousarray(
            np.asarray(inputs["enc_b2"], np.float32).reshape(128, 1)
        ),
        "dw1": np.ascontiguousarray(np.asarray(inputs["dec_w1"], np.float32)),
        "db1": np.ascontiguousarray(
            np.asarray(inputs["dec_b1"], np.float32).reshape(2, 128).T
        ),
        "dw2": np.ascontiguousarray(np.asarray(inputs["dec_w2"], np.float32)),
        "db2": np.ascontiguousarray(
            np.asarray(inputs["dec_b2"], np.float32).reshape(NCLS, 1)
        ),
        "ident": np.eye(128, dtype=np.float32),
        "iota112": np.ascontiguousarray(
            np.broadcast_to(np.arange(NSLOT, dtype=np.float32), (128, NSLOT))
        ),
        "base112": np.ascontiguousarray(
            np.broadcast_to(
                (np.arange(NSLOT) // 8 * (2 * F)).astype(np.float32),
                (128, NSLOT),
            )
        ),
        "iota128": np.ascontiguousarray(
            np.broadcast_to(np.arange(128, dtype=np.float32), (B, 128))
        ),
    }

    in_maps = []
    for core in range(NCORES):
        r0_, r1_ = core * RLOC, (core + 1) * RLOC
        shard = np.zeros((RPAD, D), dtype=np.float32)
        shard[:RLOC] = c[r0_:r1_] * scale[r0_:r1_, None]
        ct = np.ascontiguousarray(shard.T)
        zcol = np.ascontiguousarray(zb[:, core * 16:(core + 1) * 16])
        offs = np.zeros((128, 1), dtype=np.float32)
        offs[64:, 0] = F
        offs[:, 0] += core * RLOC
        m = dict(common)
        m["ct"] = ct
        m["zcol"] = zcol
        m["offs"] = offs
        in_maps.append(m)

    nc = _get_program()
    LAST_RUN = run_bass_kernel_spmd(nc, in_maps, list(range(NCORES)))
    return np.asarray(LAST_RUN.results[0]["logits"], dtype=np.float32)
